# revision 12
# baseline (speedup 1.0000x reference)
"""Trainium2 Bass kernel for NeuronLlama4VisionMLP (fused residual-add +
RMSNorm + up-proj + GELU + down-proj).

Distribution: data-parallel over the 16384 tokens -> 2048 tokens per core,
full weights replicated per core, no collectives.

Host side (cheap elementwise / repack prep):
  - h = x + residual  (this is also the module's second output)
  - per-token rsqrt(mean(h^2)+eps) scale and ln_w are folded into the
    device inputs: normed = h * s, W_up' = ln_w[:,None] * W_up
  - normed is shipped transposed+packed fp16 so the device is a pure
    matmul pipeline; the device returns out^T and b_down is added on host.

Device side per core (T=2048 tokens, H=1408, I=5632), per TB=1024 block:
    up:   psum[i_tile, tok] = sum_k W_up'[k, i_tile].T @ normed_T[k, tok]
    gelu: act[i_tile] = Gelu(psum + b_up[i_tile])       (ACT engine)
    down: psum[m_tile, chunk] = sum_i W_down[i, m_tile].T @ act[i]
    out^T[m_tile, chunk] -> HBM

All matmul operands are fp16 (f32r stationary costs ~227ns vs ~216ns per
512-col matmul due to slower LDWEIGHTS). Loads are split over both HWDGE
queues — nt on the scalar queue, wup+wdn on the sync queue — so the
prologue's critical ~4MB moves on two descriptor pipelines. Block 1's nt
load is deferred to the end of block 0's up phase: it isn't needed until
halfway through the kernel, and keeping it out of the prologue burst
avoids tripping the chip power brake early. Out DMAs ride the scalar
HWDGE queue (gpsimd software DGE is ~4x slower per packet and previously
added ~11us to the tail).
"""
import sys

sys.path.insert(0, "/opt/trn_rl_repo")

import numpy as np
import ml_dtypes
import concourse.bass as bass
from concourse import bacc
import concourse.mybir as mybir
from concourse.tile import TileContext
from concourse.bass_utils import run_bass_kernel_spmd

# Problem shape (hardcoded per contract)
B, S, H, I = 16, 1024, 1408, 5632
EPS = 1e-6
NCORES = 8
P = 128
T_CORE = (B * S) // NCORES       # 2048 tokens per core
KH = H // P                      # 11 k-tiles of H
KI = I // P                      # 44 i-tiles of I
IC = 4                           # act group tiles (ISUB i-tiles each)
ISUB = KI // IC                  # 11 i-subtiles per group
TB = 1024                        # tokens per block
NB = T_CORE // TB                # 2 blocks
NCH = TB // 512                  # 512-col matmul chunks per psum tile

F16 = mybir.dt.float16


def build_bass():
    nc = bacc.Bacc(None, target_bir_lowering=False)

    # host-packed so every DMA is HBM-contiguous with long per-partition
    # runs: nt rows are up to 22KB (k-major), wup rows 5.6KB, wdn 11.3KB
    nt = nc.declare_dram_parameter("nt", [NB, P, KH, TB], F16, isOutput=False)
    wup = nc.declare_dram_parameter("wup", [KI // 2, P, 2, KH, P], F16, isOutput=False)
    wdn = nc.declare_dram_parameter("wdn", [KH, P, IC, ISUB, P], F16, isOutput=False)
    bup = nc.declare_dram_parameter("bup", [I], mybir.dt.float32, isOutput=False)
    ot = nc.declare_dram_parameter("ot", [H, T_CORE], mybir.dt.float32, isOutput=True)

    bup2 = bup.rearrange("(i p) -> p i", p=P)         # [128, KI]

    with TileContext(nc) as tc:
        with (
            tc.tile_pool(name="const", bufs=1) as constp,
            tc.tile_pool(name="ntp", bufs=2) as ntp,
            tc.tile_pool(name="wupp", bufs=3) as wupp,
            tc.tile_pool(name="wdnp", bufs=2) as wdnp,
            tc.tile_pool(name="actp", bufs=IC) as actp,
            tc.tile_pool(name="outp", bufs=4) as outp,
            tc.tile_pool(name="psu", bufs=2, space="PSUM") as psu,
            tc.tile_pool(name="psd", bufs=4, space="PSUM") as psd,
        ):
            bup_sb = constp.tile([P, KI], mybir.dt.float32)
            nc.gpsimd.dma_start(out=bup_sb[:], in_=bup2)

            # block 0's nt leads the scalar HWDGE queue; PE consumes k in
            # order, so 3 slices let the first matmul chain start after
            # ~0.6MB, not ~3.2MB
            ntt0 = ntp.tile([P, KH, TB], F16, tag="ntb", name="nt0")
            nc.scalar.dma_start(out=ntt0[:, 0], in_=nt[0][:, 0])
            nc.scalar.dma_start(out=ntt0[:, 1:4], in_=nt[0][:, 1:4])
            nc.scalar.dma_start(out=ntt0[:, 4:7], in_=nt[0][:, 4:7])
            nc.scalar.dma_start(out=ntt0[:, 7:], in_=nt[0][:, 7:])
            ntts = [ntt0]

            for b in range(NB):
                ntt = ntts[b]

                # ---- up projection + gelu ----
                act_g = []
                wup_tiles = {}

                def load_wup(ip, split=False):
                    t = wupp.tile([P, 2, KH, P], F16, tag="wup", name=f"wup{ip}")
                    if split:
                        nc.sync.dma_start(out=t[:, 0], in_=wup[ip][:, 0])
                        nc.sync.dma_start(out=t[:, 1], in_=wup[ip][:, 1])
                    else:
                        nc.sync.dma_start(out=t[:], in_=wup[ip])
                    wup_tiles[ip] = t

                load_wup(0, split=(b == 0))
                load_wup(1)
                for ip in range(KI // 2):
                    # prefetch weight tiles two ahead of this ip's
                    # activations so their triggers aren't queued behind
                    # them and the transfers lead the PE by ~9us
                    if ip + 2 < KI // 2:
                        load_wup(ip + 2)
                    wupb = wup_tiles.pop(ip)
                    for half in range(2):
                        i = 2 * ip + half
                        g, s = divmod(i, ISUB)
                        if s == 0:
                            act_g.append(
                                actp.tile(
                                    [P, ISUB, TB], F16, tag="act", name=f"act{b}_{g}"
                                )
                            )
                        ps = psu.tile([P, TB], mybir.dt.float32, tag="psu")
                        for c in range(NCH):
                            cs = slice(c * 512, (c + 1) * 512)
                            for k in range(KH):
                                nc.tensor.matmul(
                                    ps[:, cs],
                                    wupb[:, half, k],
                                    ntt[:, k, cs],
                                    start=(k == 0),
                                    stop=(k == KH - 1),
                                )
                        nc.scalar.activation(
                            act_g[g][:, s],
                            ps[:],
                            mybir.ActivationFunctionType.Gelu,
                            bias=bup_sb[:, i : i + 1],
                            scale=1.0,
                        )

                # prefetch next block's nt now: the trigger fires right
                # after this block's last activation and streams during
                # the down phase, well before it's consumed
                if b + 1 < NB:
                    ntt_n = ntp.tile([P, KH, TB], F16, tag="ntb", name=f"nt{b+1}")
                    nc.scalar.dma_start(out=ntt_n[:], in_=nt[b + 1])
                    ntts.append(ntt_n)

                # ---- down projection ----
                for m in range(KH):
                    wdnb = wdnp.tile([P, IC, ISUB, P], F16, tag="wdn")
                    nc.sync.dma_start(out=wdnb[:], in_=wdn[m])
                    for c in range(NCH):
                        cs = slice(c * 512, (c + 1) * 512)
                        ps2 = psd.tile([P, 512], mybir.dt.float32, tag="psd")
                        for i in range(KI):
                            g, s = divmod(i, ISUB)
                            nc.tensor.matmul(
                                ps2[:],
                                wdnb[:, g, s],
                                act_g[g][:, s, cs],
                                start=(i == 0),
                                stop=(i == KI - 1),
                            )
                        osb = outp.tile([P, 512], mybir.dt.float32, tag="osb")
                        nc.vector.tensor_copy(out=osb[:], in_=ps2[:])
                        nc.scalar.dma_start(
                            out=ot[
                                m * P : (m + 1) * P,
                                b * TB + c * 512 : b * TB + (c + 1) * 512,
                            ],
                            in_=osb[:],
                        )
    nc.compile()
    return nc


_CACHED = {}


def _get_nc():
    if "nc" not in _CACHED:
        _CACHED["nc"] = build_bass()
    return _CACHED["nc"]


def _prep_host(x, residual, ln_w, W_up, b_up, W_down):
    """Host-side prep: h, packed normed^T per core, repacked fp16 weights."""
    h = x + residual                                   # [B,S,H] f32
    hf = h.reshape(-1, H)                              # [16384, H]
    var = np.mean(np.square(hf), axis=-1)              # f32
    s = 1.0 / np.sqrt(var + EPS)                       # f32
    normed = hf * s[:, None]                           # f32 (ln_w folded into W)

    Wup_p = W_up * ln_w[:, None]                       # [H, I]
    # wup[ip, p, half, k, il] = Wup_p[k*128+p, (2*ip+half)*128+il]
    WUP = np.ascontiguousarray(
        Wup_p.reshape(KH, P, KI // 2, 2, P).transpose(2, 1, 3, 0, 4)
    ).astype(np.float16)                               # [KI/2,P,2,KH,P]
    # wdn[m, p, ic, isub, c] = W_down[(ic*ISUB+isub)*128+p, m*128+c]
    WDN = np.ascontiguousarray(
        W_down.reshape(IC, ISUB, P, KH, P).transpose(3, 2, 0, 1, 4)
    ).astype(np.float16)                               # [KH,P,IC,ISUB,P]

    in_maps = []
    for cc in range(NCORES):
        ntc = normed[cc * T_CORE : (cc + 1) * T_CORE].T  # [H, T_CORE]
        # nt[b, p, k, tt] = ntc[k*128+p, b*TB+tt]
        ntp = np.ascontiguousarray(
            ntc.reshape(KH, P, NB, TB).transpose(2, 1, 0, 3)
        ).astype(np.float16)                           # [NB,P,KH,TB]
        in_maps.append(
            {"nt": ntp, "wup": WUP, "wdn": WDN, "bup": b_up.astype(np.float32)}
        )
    return h, in_maps


_RESET_DONE = {}


def _maybe_reset_device():
    """Best-effort terminal NRT reset so a previously wedged device can't
    hang the run. No-op when the axon .so or symbol is unavailable."""
    if _RESET_DONE:
        return
    _RESET_DONE["done"] = True
    try:
        import ctypes
        import jax

        jax.devices()
        lib = ctypes.CDLL("/opt/axon/libaxon_pjrt.so")
        if hasattr(lib, "axon_reset"):
            lib.axon_reset.restype = ctypes.c_int64
            lib.axon_reset()
    except Exception:
        pass


def _run(in_maps, **kw):
    _maybe_reset_device()
    nc = _get_nc()
    return run_bass_kernel_spmd(nc, in_maps, core_ids=list(range(NCORES)), **kw)


def _assemble(results, b_down):
    outs = [r["ot"].T for r in results]                # each [T_CORE, H]
    out = np.concatenate(outs, axis=0).reshape(B, S, H)
    out = out + b_down.astype(np.float32)
    return out


def kernel(x, residual, ln_w, W_up, b_up, W_down, b_down):
    x = np.asarray(x, dtype=np.float32)
    residual = np.asarray(residual, dtype=np.float32)
    ln_w = np.asarray(ln_w, dtype=np.float32)
    W_up = np.asarray(W_up, dtype=np.float32)
    b_up = np.asarray(b_up, dtype=np.float32)
    W_down = np.asarray(W_down, dtype=np.float32)
    b_down = np.asarray(b_down, dtype=np.float32)

    h, in_maps = _prep_host(x, residual, ln_w, W_up, b_up, W_down)
    res = _run(in_maps)
    out = _assemble(res.results, b_down)
    return out, h


def kernel_traced(x, residual, ln_w, W_up, b_up, W_down, b_down, **kw):
    """Like kernel() but with NTFF tracing; returns ((out, h), results)."""
    h, in_maps = _prep_host(
        np.asarray(x, np.float32),
        np.asarray(residual, np.float32),
        np.asarray(ln_w, np.float32),
        np.asarray(W_up, np.float32),
        np.asarray(b_up, np.float32),
        np.asarray(W_down, np.float32),
    )
    res = _run(in_maps, trace=True, **kw)
    out = _assemble(res.results, np.asarray(b_down, np.float32))
    return (out, h), res


# revision 15
# speedup vs baseline: 1.0019x; 1.0019x over previous
"""Trainium2 Bass kernel for NeuronLlama4VisionMLP (fused residual-add +
RMSNorm + up-proj + GELU + down-proj).

Distribution: data-parallel over the 16384 tokens -> 2048 tokens per core,
full weights replicated per core, no collectives.

Host side (cheap elementwise / repack prep):
  - h = x + residual  (this is also the module's second output)
  - per-token rsqrt(mean(h^2)+eps) scale and ln_w are folded into the
    device inputs: normed = h * s, W_up' = ln_w[:,None] * W_up
  - normed is shipped transposed+packed fp16 so the device is a pure
    matmul pipeline; the device returns out^T and b_down is added on host.

Device side per core (T=2048 tokens, H=1408, I=5632), per TB=1024 block:
    up:   psum[i_tile, tok] = sum_k W_up'[k, i_tile].T @ normed_T[k, tok]
    gelu: act[i_tile] = Gelu(psum + b_up[i_tile])       (ACT engine)
    down: psum[m_tile, chunk] = sum_i W_down[i, m_tile].T @ act[i]
    out^T[m_tile, chunk] -> HBM

All matmul operands are fp16 (f32r stationary costs ~227ns vs ~216ns per
512-col matmul due to slower LDWEIGHTS). Loads are split over both HWDGE
queues — nt on the scalar queue, wup+wdn on the sync queue — so the
prologue's critical ~4MB moves on two descriptor pipelines. Block 1's nt
load is deferred to the end of block 0's up phase: it isn't needed until
halfway through the kernel, and keeping it out of the prologue burst
avoids tripping the chip power brake early. Out DMAs ride the scalar
HWDGE queue (gpsimd software DGE is ~4x slower per packet and previously
added ~11us to the tail).
"""
import sys

sys.path.insert(0, "/opt/trn_rl_repo")

import numpy as np
import ml_dtypes
import concourse.bass as bass
from concourse import bacc
import concourse.mybir as mybir
from concourse.tile import TileContext
from concourse.bass_utils import run_bass_kernel_spmd

# Problem shape (hardcoded per contract)
B, S, H, I = 16, 1024, 1408, 5632
EPS = 1e-6
NCORES = 8
P = 128
T_CORE = (B * S) // NCORES       # 2048 tokens per core
KH = H // P                      # 11 k-tiles of H
KI = I // P                      # 44 i-tiles of I
IC = 4                           # act group tiles (ISUB i-tiles each)
ISUB = KI // IC                  # 11 i-subtiles per group
TB = 1024                        # tokens per block
NB = T_CORE // TB                # 2 blocks
NCH = TB // 512                  # 512-col matmul chunks per psum tile

F16 = mybir.dt.float16


def build_bass():
    nc = bacc.Bacc(None, target_bir_lowering=False)

    # host-packed so every DMA is HBM-contiguous with long per-partition
    # runs: nt rows are up to 22KB (k-major), wup rows 5.6KB, wdn 11.3KB
    nt = nc.declare_dram_parameter("nt", [NB, P, KH, TB], F16, isOutput=False)
    wup = nc.declare_dram_parameter("wup", [KI // 2, P, 2, KH, P], F16, isOutput=False)
    wdn = nc.declare_dram_parameter("wdn", [KH, P, IC, ISUB, P], F16, isOutput=False)
    bup = nc.declare_dram_parameter("bup", [I], mybir.dt.float32, isOutput=False)
    ot = nc.declare_dram_parameter("ot", [H, T_CORE], mybir.dt.float32, isOutput=True)

    bup2 = bup.rearrange("(i p) -> p i", p=P)         # [128, KI]

    with TileContext(nc) as tc:
        with (
            tc.tile_pool(name="const", bufs=1) as constp,
            tc.tile_pool(name="ntp", bufs=2) as ntp,
            tc.tile_pool(name="wupp", bufs=2) as wupp,
            tc.tile_pool(name="wdnp", bufs=2) as wdnp,
            tc.tile_pool(name="actp", bufs=IC) as actp,
            tc.tile_pool(name="outp", bufs=4) as outp,
            tc.tile_pool(name="psu", bufs=2, space="PSUM") as psu,
            tc.tile_pool(name="psd", bufs=4, space="PSUM") as psd,
        ):
            bup_sb = constp.tile([P, KI], mybir.dt.float32)
            nc.gpsimd.dma_start(out=bup_sb[:], in_=bup2)

            # block 0's nt leads the scalar HWDGE queue; PE consumes k in
            # order, so 3 slices let the first matmul chain start after
            # ~0.6MB, not ~3.2MB
            ntt0 = ntp.tile([P, KH, TB], F16, tag="ntb", name="nt0")
            nc.scalar.dma_start(out=ntt0[:, 0], in_=nt[0][:, 0])
            nc.scalar.dma_start(out=ntt0[:, 1:4], in_=nt[0][:, 1:4])
            nc.scalar.dma_start(out=ntt0[:, 4:], in_=nt[0][:, 4:])
            ntts = [ntt0]

            for b in range(NB):
                ntt = ntts[b]

                # ---- up projection + gelu ----
                act_g = []
                wup_tiles = {}

                def load_wup(ip, split=False):
                    t = wupp.tile([P, 2, KH, P], F16, tag="wup", name=f"wup{ip}")
                    if split:
                        nc.sync.dma_start(out=t[:, 0], in_=wup[ip][:, 0])
                        nc.sync.dma_start(out=t[:, 1], in_=wup[ip][:, 1])
                    else:
                        nc.sync.dma_start(out=t[:], in_=wup[ip])
                    wup_tiles[ip] = t

                load_wup(0, split=(b == 0))
                for ip in range(KI // 2):
                    # prefetch next weight tile ahead of this ip's
                    # activations so its trigger isn't queued behind them
                    if ip + 1 < KI // 2:
                        load_wup(ip + 1)
                    wupb = wup_tiles.pop(ip)
                    for half in range(2):
                        i = 2 * ip + half
                        g, s = divmod(i, ISUB)
                        if s == 0:
                            act_g.append(
                                actp.tile(
                                    [P, ISUB, TB], F16, tag="act", name=f"act{b}_{g}"
                                )
                            )
                        ps = psu.tile([P, TB], mybir.dt.float32, tag="psu")
                        for c in range(NCH):
                            cs = slice(c * 512, (c + 1) * 512)
                            for k in range(KH):
                                nc.tensor.matmul(
                                    ps[:, cs],
                                    wupb[:, half, k],
                                    ntt[:, k, cs],
                                    start=(k == 0),
                                    stop=(k == KH - 1),
                                )
                        nc.scalar.activation(
                            act_g[g][:, s],
                            ps[:],
                            mybir.ActivationFunctionType.Gelu,
                            bias=bup_sb[:, i : i + 1],
                            scale=1.0,
                        )

                # prefetch next block's nt now: the trigger fires right
                # after this block's last activation and streams during
                # the down phase, well before it's consumed
                if b + 1 < NB:
                    ntt_n = ntp.tile([P, KH, TB], F16, tag="ntb", name=f"nt{b+1}")
                    nc.scalar.dma_start(out=ntt_n[:], in_=nt[b + 1])
                    ntts.append(ntt_n)

                # ---- down projection ----
                for m in range(KH):
                    wdnb = wdnp.tile([P, IC, ISUB, P], F16, tag="wdn")
                    nc.sync.dma_start(out=wdnb[:], in_=wdn[m])
                    for c in range(NCH):
                        cs = slice(c * 512, (c + 1) * 512)
                        ps2 = psd.tile([P, 512], mybir.dt.float32, tag="psd")
                        for i in range(KI):
                            g, s = divmod(i, ISUB)
                            nc.tensor.matmul(
                                ps2[:],
                                wdnb[:, g, s],
                                act_g[g][:, s, cs],
                                start=(i == 0),
                                stop=(i == KI - 1),
                            )
                        osb = outp.tile([P, 512], mybir.dt.float32, tag="osb")
                        nc.vector.tensor_copy(out=osb[:], in_=ps2[:])
                        nc.scalar.dma_start(
                            out=ot[
                                m * P : (m + 1) * P,
                                b * TB + c * 512 : b * TB + (c + 1) * 512,
                            ],
                            in_=osb[:],
                        )
    nc.compile()
    return nc


_CACHED = {}


def _get_nc():
    if "nc" not in _CACHED:
        _CACHED["nc"] = build_bass()
    return _CACHED["nc"]


def _prep_host(x, residual, ln_w, W_up, b_up, W_down):
    """Host-side prep: h, packed normed^T per core, repacked fp16 weights."""
    h = x + residual                                   # [B,S,H] f32
    hf = h.reshape(-1, H)                              # [16384, H]
    var = np.mean(np.square(hf), axis=-1)              # f32
    s = 1.0 / np.sqrt(var + EPS)                       # f32
    normed = hf * s[:, None]                           # f32 (ln_w folded into W)

    Wup_p = W_up * ln_w[:, None]                       # [H, I]
    # wup[ip, p, half, k, il] = Wup_p[k*128+p, (2*ip+half)*128+il]
    WUP = np.ascontiguousarray(
        Wup_p.reshape(KH, P, KI // 2, 2, P).transpose(2, 1, 3, 0, 4)
    ).astype(np.float16)                               # [KI/2,P,2,KH,P]
    # wdn[m, p, ic, isub, c] = W_down[(ic*ISUB+isub)*128+p, m*128+c]
    WDN = np.ascontiguousarray(
        W_down.reshape(IC, ISUB, P, KH, P).transpose(3, 2, 0, 1, 4)
    ).astype(np.float16)                               # [KH,P,IC,ISUB,P]

    in_maps = []
    for cc in range(NCORES):
        ntc = normed[cc * T_CORE : (cc + 1) * T_CORE].T  # [H, T_CORE]
        # nt[b, p, k, tt] = ntc[k*128+p, b*TB+tt]
        ntp = np.ascontiguousarray(
            ntc.reshape(KH, P, NB, TB).transpose(2, 1, 0, 3)
        ).astype(np.float16)                           # [NB,P,KH,TB]
        in_maps.append(
            {"nt": ntp, "wup": WUP, "wdn": WDN, "bup": b_up.astype(np.float32)}
        )
    return h, in_maps


_RESET_DONE = {}


def _maybe_reset_device():
    """Best-effort terminal NRT reset so a previously wedged device can't
    hang the run. No-op when the axon .so or symbol is unavailable."""
    if _RESET_DONE:
        return
    _RESET_DONE["done"] = True
    try:
        import ctypes
        import jax

        jax.devices()
        lib = ctypes.CDLL("/opt/axon/libaxon_pjrt.so")
        if hasattr(lib, "axon_reset"):
            lib.axon_reset.restype = ctypes.c_int64
            lib.axon_reset()
    except Exception:
        pass


def _run(in_maps, **kw):
    _maybe_reset_device()
    nc = _get_nc()
    return run_bass_kernel_spmd(nc, in_maps, core_ids=list(range(NCORES)), **kw)


def _assemble(results, b_down):
    outs = [r["ot"].T for r in results]                # each [T_CORE, H]
    out = np.concatenate(outs, axis=0).reshape(B, S, H)
    out = out + b_down.astype(np.float32)
    return out


def kernel(x, residual, ln_w, W_up, b_up, W_down, b_down):
    x = np.asarray(x, dtype=np.float32)
    residual = np.asarray(residual, dtype=np.float32)
    ln_w = np.asarray(ln_w, dtype=np.float32)
    W_up = np.asarray(W_up, dtype=np.float32)
    b_up = np.asarray(b_up, dtype=np.float32)
    W_down = np.asarray(W_down, dtype=np.float32)
    b_down = np.asarray(b_down, dtype=np.float32)

    h, in_maps = _prep_host(x, residual, ln_w, W_up, b_up, W_down)
    res = _run(in_maps)
    out = _assemble(res.results, b_down)
    return out, h


def kernel_traced(x, residual, ln_w, W_up, b_up, W_down, b_down, **kw):
    """Like kernel() but with NTFF tracing; returns ((out, h), results)."""
    h, in_maps = _prep_host(
        np.asarray(x, np.float32),
        np.asarray(residual, np.float32),
        np.asarray(ln_w, np.float32),
        np.asarray(W_up, np.float32),
        np.asarray(b_up, np.float32),
        np.asarray(W_down, np.float32),
    )
    res = _run(in_maps, trace=True, **kw)
    out = _assemble(res.results, np.asarray(b_down, np.float32))
    return (out, h), res
